# revision 26
# baseline (speedup 1.0000x reference)
"""Trainium2 Bass kernel for CompressedCausalAttention.

Reference computation (S=1024, B=4, C=1024, H=16, CC=64):
    qkv = (x + pe) @ Wqkv.T + bqkv
    q, k, v = split(qkv); reshape to [s, b, H, CC]
    qk = einsum('sbhc,tbhc->stbh', q, k) / sqrt(CC)
    mask = content_mask[:,:,:,None] | padding_mask[None,:,:,None]
    p = softmax(where(mask, -inf, qk), axis=1)
    out = einsum('stbh,tbhc->sbhc', p, v).reshape(s,b,c) @ Wo.T + bo

Sharding: 8 cores = (batch b in 0..3) x (head-group hg in 0..1, 8 heads each).
Each core projects q/k/v for its 8 heads of its batch, computes attention with
scores in transposed [t, s] layout (softmax row-sum comes free from an
appended ones-column in V; no max-subtraction needed at these magnitudes),
pair-AllGathers the per-head attention outputs within a batch, and computes
the output projection for its half of the output channels (host-sliced Wo
keeps the SPMD program identical on all cores).
"""

import os
import sys
import types

import ml_dtypes
import numpy as np

_SO_PATH = "/opt/axon/libaxon_pjrt.so"


def _install_ntff_shim():
    """Make `antenv.axon_hooks` importable so trace=True works under axon."""
    try:
        from antenv.axon_hooks import set_axon_ntff_profile_hook  # noqa: F401

        return
    except ImportError:
        pass
    try:
        import antenv
        import trn_agent_boot.trn_boot as tb
    except ImportError:
        return
    mod = types.ModuleType("antenv.axon_hooks")
    _hook = [None]
    mod.set_axon_ntff_profile_hook = lambda h: _hook.__setitem__(0, h)
    mod.get_axon_ntff_profile_hook = lambda: _hook[0]
    sys.modules["antenv.axon_hooks"] = mod
    antenv.axon_hooks = mod
    if os.path.exists(_SO_PATH):
        mod.set_axon_ntff_profile_hook(tb._ntff_profile_via_ctypes(_SO_PATH))


_install_ntff_shim()

import concourse.bass as bass  # noqa: E402
import concourse.tile as tile  # noqa: E402
from concourse import bacc, mybir  # noqa: E402
from concourse.bass_utils import run_bass_kernel_spmd  # noqa: E402

S = 1024
B = 4
C = 1024
H = 16
CC = 64
HG = 8  # heads per core
F = HG * CC  # 512 features per core for each of q/k/v
P = 128
NQ = S // P  # 8 query tiles
NT = S // P  # 8 key tiles
KT = C // P  # 8 contraction tiles
TEMP = 1.0 / 8.0

DTB = mybir.dt.bfloat16
DTF = mybir.dt.float32
BF16 = ml_dtypes.bfloat16

N_CORES = 8
PAIRS = [[2 * b, 2 * b + 1] for b in range(B)]

_NC_CACHE = {}
WARMUP_MMS = 120
LAST_RESULT = None  # BassKernelResults of the most recent run (for profiling)


def _build(causal: bool, tmin: int):
    """Build the SPMD program. `causal`: triangular block skipping + tril
    mask on diagonal score blocks; padding is folded into V (padded rows and
    the denominator column are zeroed). Otherwise a full [t, s] 0/1 mask
    input is applied per score block."""
    nc = bacc.Bacc("TRN2", target_bir_lowering=False, debug=False,
                   num_devices=N_CORES)

    xT = nc.dram_tensor("xT", [C, S], DTB, kind="ExternalInput")
    peT = nc.dram_tensor("peT", [C, S], DTB, kind="ExternalInput")
    wqkvT = nc.dram_tensor("wqkvT", [C, 3 * F], DTB, kind="ExternalInput")
    woT = nc.dram_tensor("woT", [C, C // 2], DTB, kind="ExternalInput")
    bq_row = nc.dram_tensor("bq_row", [1, F], DTB, kind="ExternalInput")
    bk_row = nc.dram_tensor("bk_row", [1, F], DTB, kind="ExternalInput")
    bv_row = nc.dram_tensor("bv_row", [1, F], DTB, kind="ExternalInput")
    bo_row = nc.dram_tensor("bo_row", [1, C // 2], DTB, kind="ExternalInput")
    if causal:
        pad01 = nc.dram_tensor("pad01", [P, NT], DTF, kind="ExternalInput")
        pad_colb = nc.dram_tensor("pad_colb", [P, NT, HG], DTB,
                                  kind="ExternalInput")
        maskT = None
    else:
        pad01 = None
        maskT = nc.dram_tensor("maskT", [P, NT, S], DTB, kind="ExternalInput")
    out_h = nc.dram_tensor("out", [S, C // 2], DTF, kind="ExternalOutput")

    # chunked pair-AllGather buffers (token quarters)
    NAG = 4
    AGW = S // NAG
    ag_in = [nc.dram_tensor(f"ag_in{i}", [F, AGW], DTB) for i in range(NAG)]
    ag_out = [nc.dram_tensor(f"ag_out{i}", [2 * F, AGW], DTB)
              for i in range(NAG)]

    tril_np = np.triu(np.ones((P, P))).astype(BF16)  # keep t <= s
    tril_dram = nc.inline_tensor(tril_np, name="tril_const")
    ident_dram = nc.inline_tensor(np.eye(P).astype(BF16), name="ident_const")

    cpy = mybir.ActivationFunctionType.Copy
    from contextlib import ExitStack

    with tile.TileContext(nc) as tc, ExitStack() as ctx:
        cpool = ctx.enter_context(tc.tile_pool(name="const", bufs=1))
        pp = ctx.enter_context(tc.tile_pool(name="persist", bufs=1))

        xpe = pp.tile([P, KT, S], DTB)
        wq_b = pp.tile([P, KT, 3 * F], DTB)
        wo_b = pp.tile([P, KT, C // 2], DTB)
        q_t = pp.tile([P, F // P, S], DTB)
        k_t = pp.tile([P, F // P, S], DTB)
        v_t = pp.tile([P, NT, HG * (CC + 1)], DTB)
        attn_l = pp.tile([P, F // P, S], DTB)     # attn^T local [hc, tok]
        attn_f = pp.tile([P, KT, S], DTB)         # attn^T both head-groups

        tril = cpool.tile([P, P], DTB)
        nc.sync.dma_start(tril[:], tril_dram[:])
        ident = cpool.tile([P, P], DTB)
        nc.sync.dma_start(ident[:], ident_dram[:])
        ones_r = cpool.tile([1, P], DTB)
        nc.vector.memset(ones_r[:], 1.0)
        ones_half = cpool.tile([1, S // 2], DTB)
        nc.vector.memset(ones_half[:], 1.0)
        warm_rhs = cpool.tile([P, 256], DTB)
        nc.vector.memset(warm_rhs[:], 0.0)
        bq_t = cpool.tile([1, F], DTB)
        nc.sync.dma_start(bq_t[:], bq_row[:])
        bk_t = cpool.tile([1, F], DTB)
        nc.sync.dma_start(bk_t[:], bk_row[:])
        bv_t = cpool.tile([1, F], DTB)
        nc.sync.dma_start(bv_t[:], bv_row[:])
        bo_t = cpool.tile([1, C // 2], DTB)
        nc.sync.dma_start(bo_t[:], bo_row[:])
        if causal:
            pad_t = cpool.tile([P, NT], DTF)
            nc.sync.dma_start(pad_t[:], pad01[:])
            pad_cb = cpool.tile([P, NT, HG], DTB)
            nc.sync.dma_start(pad_cb[:], pad_colb[:])
        else:
            m_t = pp.tile([P, NT, S], DTB)
            for tj in range(NT):
                nc.sync.dma_start(m_t[:, tj, :], maskT[:, tj, :])

        # ---- PE warmup: dummy matmuls fill the input-DMA window so the
        # HAM clock gate is released before the projections start ----
        with tc.tile_pool(name="warmps", bufs=1, space="PSUM") as warmps:
            wps = warmps.tile([P, 256], DTF)
            for _ in range(WARMUP_MMS):
                nc.tensor.matmul(wps[:], ident[:], warm_rhs[:])

        # ---- load + convert inputs to bf16 (x/pe and qkv weights) ----
        stage = ctx.enter_context(tc.tile_pool(name="stage", bufs=4))
        for kt in range(KT):
            nc.sync.dma_start(xpe[:, kt, :], xT[P * kt:P * (kt + 1), :])
            ps_ = stage.tile([P, S], DTB, tag="ps")
            nc.gpsimd.dma_start(ps_[:], peT[P * kt:P * (kt + 1), :])
            nc.scalar.dma_start(wq_b[:, kt, :], wqkvT[P * kt:P * (kt + 1), :])
            nc.vector.tensor_add(xpe[:, kt, :], xpe[:, kt, :], ps_[:])
        for kt in range(KT):
            nc.gpsimd.dma_start(wo_b[:, kt, :], woT[P * kt:P * (kt + 1), :])

        # ---- projections, in kt-outer "waves" so the matmuls stream right
        # behind the weight DMAs. Emission of attention score chunks is WOVEN
        # with slices of projection / output-projection matmuls: the PE
        # instruction stream is strictly in-order, so each score->exp
        # ping-pong stall is filled with independent dense matmul work ----
        v3 = v_t[:].rearrange("p n (h c) -> p n h c", c=CC + 1)
        if causal:
            nc.vector.tensor_copy(v3[:, :, :, CC], pad_cb[:])
        else:
            nc.vector.memset(v3[:, :, :, CC], 1.0)

        ep = ctx.enter_context(tc.tile_pool(name="ep", bufs=2))
        ptp = ctx.enter_context(tc.tile_pool(name="ptp", bufs=3))
        osbp = ctx.enter_context(tc.tile_pool(name="osb", bufs=3))

        ops_pool = ctx.enter_context(tc.tile_pool(name="ops", bufs=1,
                                                  space="PSUM"))
        phase1 = ExitStack()
        projps = phase1.enter_context(tc.tile_pool(name="projps", bufs=1,
                                                   space="PSUM"))
        scps_a = phase1.enter_context(tc.tile_pool(name="scpsa", bufs=1,
                                                   space="PSUM"))

        def qk_wave(which, nh):
            # generator: project 4 feature tiles of q (which=0) / k (which=1)
            tiles = [projps.tile([P, S // 2], DTF, tag=f"pj{i}",
                                 name=f"psq{i}") for i in range(4)]
            for kt in range(KT):
                for ft in range(4):
                    nc.tensor.matmul(
                        tiles[ft][:],
                        wq_b[:, kt, F * which + P * ft:F * which + P * (ft + 1)],
                        xpe[:, kt, (S // 2) * nh:(S // 2) * (nh + 1)],
                        start=(kt == 0), stop=False,
                    )
                yield
            bias_t = bq_t if which == 0 else bk_t
            dst = q_t if which == 0 else k_t
            for ft in range(4):
                nc.tensor.matmul(tiles[ft][:], bias_t[:, P * ft:P * (ft + 1)],
                                 ones_half[:, :], start=False, stop=True)
                nc.vector.tensor_copy(
                    dst[:, ft, (S // 2) * nh:(S // 2) * (nh + 1)], tiles[ft][:])

        def v_wave(half):
            tiles = [projps.tile([P, F], DTF, tag=f"pj{i}", name=f"psv{i}")
                     for i in range(4)]
            for kt in range(KT):
                for i in range(4):
                    tt = 4 * half + i
                    nc.tensor.matmul(
                        tiles[i][:], xpe[:, kt, P * tt:P * (tt + 1)],
                        wq_b[:, kt, 2 * F:3 * F],
                        start=(kt == 0), stop=False,
                    )
                yield
            for i in range(4):
                tt = 4 * half + i
                nc.tensor.matmul(tiles[i][:], ones_r[:, :], bv_t[:, :],
                                 start=False, stop=True)
                if causal:
                    nc.vector.tensor_scalar_mul(
                        v3[:, tt, :, 0:CC],
                        tiles[i][:].rearrange("p (h c) -> p h c", c=CC),
                        pad_t[:, tt:tt + 1],
                    )
                else:
                    nc.vector.tensor_copy(
                        v3[:, tt, :, 0:CC],
                        tiles[i][:].rearrange("p (h c) -> p h c", c=CC),
                    )

        def attention_pair(qp, scpool, tppool, tptag, ctj):
            # generator: scores/exp chunks (ctj key tiles each) yield between
            # chunks so filler matmuls can be woven into the PE stream
            q0, q1 = 2 * qp, 2 * qp + 1
            n_t = q1 + 1 if causal else NT
            pts = []
            for hp in range(HG // 2):
                ft = hp
                pt = ptp.tile([P, 2, NT, 2 * P], DTB, tag="pt", name="pt")
                pts.append(pt)
                for c0 in range(0, n_t, ctj):
                    cn = min(ctj, n_t - c0)
                    scp = scpool.tile([P, 2, ctj, 2 * P], DTF, tag="scp",
                                      name="scp")
                    for tj in range(c0, c0 + cn):
                        nc.tensor.matmul(
                            scp[:, 0, tj - c0, :],
                            k_t[0:CC, ft, P * tj:P * (tj + 1)],
                            q_t[0:CC, ft, 2 * P * qp:2 * P * (qp + 1)],
                        )
                        nc.tensor.matmul(
                            scp[:, 1, tj - c0, :],
                            k_t[CC:P, ft, P * tj:P * (tj + 1)],
                            q_t[CC:P, ft, 2 * P * qp:2 * P * (qp + 1)],
                        )
                    nc.scalar.activation(
                        pt[:, :, c0:c0 + cn, :], scp[:, :, 0:cn, :],
                        mybir.ActivationFunctionType.Exp, scale=TEMP)
                    yield
            for iq, qi in enumerate((q0, q1)):
                nt_i = qi + 1 if causal else NT
                out_ab = [ops_pool.tile([P, HG // 2, CC + 1], DTF,
                                        tag=f"out{x}", name=f"out_ab{x}")
                          for x in range(2)]
                for hp in range(HG // 2):
                    pt = pts[hp]
                    if causal:
                        nc.vector.tensor_mul(
                            pt[:, :, qi, P * iq:P * (iq + 1)],
                            pt[:, :, qi, P * iq:P * (iq + 1)], tril2[:])
                    else:
                        for tj in range(nt_i):
                            nc.vector.tensor_mul(
                                pt[:, :, tj, P * iq:P * (iq + 1)],
                                pt[:, :, tj, P * iq:P * (iq + 1)],
                                m2_t[:, :, tj, P * qi:P * (qi + 1)])
                    for x, h in ((0, 2 * hp), (1, 2 * hp + 1)):
                        for tj in range(nt_i):
                            nc.tensor.matmul(
                                out_ab[h // 4][:, h % 4, :],
                                pt[:, x, tj, P * iq:P * (iq + 1)],
                                v_t[:, tj, (CC + 1) * h:(CC + 1) * (h + 1)],
                                start=(tj == 0), stop=(tj == nt_i - 1),
                            )
                    yield
                # normalization epilogue for this query tile
                rec = ep.tile([P, HG], DTF, tag="rec", name="rec")
                for x in range(2):
                    nc.vector.reciprocal(rec[:, 4 * x:4 * (x + 1)],
                                         out_ab[x][:, :, CC])
                attn_s = ep.tile([P, F], DTB, tag="attn_s", name="attn_s")
                for h in range(HG):
                    nc.vector.tensor_scalar_mul(
                        attn_s[:, CC * h:CC * (h + 1)],
                        out_ab[h // 4][:, h % 4, 0:CC],
                        rec[:, h:h + 1],
                    )
                tp = tppool.tile([P, HG // 2, P], DTB, tag=tptag, name="tp")
                for hp in range(HG // 2):
                    nc.tensor.transpose(tp[:, hp, :],
                                        attn_s[:, P * hp:P * (hp + 1)],
                                        ident[:])
                    nc.vector.tensor_copy(attn_l[:, hp, P * qi:P * (qi + 1)],
                                          tp[:, hp, :])
                yield

        def ag_chunk(i):
            for ft in range(F // P):
                nc.sync.dma_start(
                    ag_in[i][P * ft:P * (ft + 1), :],
                    attn_l[:, ft, AGW * i:AGW * (i + 1)])
            nc.gpsimd.collective_compute(
                "AllGather", mybir.AluOpType.bypass, replica_groups=PAIRS,
                ins=[ag_in[i][:]], outs=[ag_out[i][:]],
            )
            for kc in range(KT):
                nc.gpsimd.dma_start(
                    attn_f[:, kc, AGW * i:AGW * (i + 1)],
                    ag_out[i][P * kc:P * (kc + 1), :])

        def out_proj(mt, pool, tag):
            psf = pool.tile([P, C // 2], DTF, tag=tag, name="psf")
            for kc in range(KT):
                nc.tensor.matmul(
                    psf[:], attn_f[:, kc, P * mt:P * (mt + 1)],
                    wo_b[:, kc, :],
                    start=(kc == 0), stop=False,
                )
                if kc == 3:
                    yield
            nc.tensor.matmul(psf[:], ones_r[:, :], bo_t[:, :],
                             start=False, stop=True)
            osb = osbp.tile([P, C // 2], DTF, tag="osb", name="osb")
            nc.scalar.copy(osb[:], psf[:])
            nc.sync.dma_start(out_h[P * mt:P * (mt + 1), :], osb[:])
            yield

        def weave(main_gen, fillers):
            """Run main_gen; after each of its yields, advance the current
            filler generator by one step."""
            for _ in main_gen:
                while fillers:
                    try:
                        next(fillers[0])
                        break
                    except StopIteration:
                        fillers.pop(0)
            for fg in fillers:
                for _ in fg:
                    pass
            fillers.clear()

        def run(gen):
            for _ in gen:
                pass

        # doubled tril (for masking both heads of a pair in one op)
        tril2 = cpool.tile([P, 2, P], DTB)
        for x in range(2):
            nc.vector.tensor_copy(tril2[:, x, :], tril[:])
        if not causal:
            m2_t = pp.tile([P, 2, NT, S], DTB)
            for x in range(2):
                nc.vector.tensor_copy(m2_t[:, x, :, :], m_t[:])

        # phase 1: q/k/v token-half 0 dense; then pairs 0-1 woven with the
        # remaining projection waves
        run(qk_wave(0, 0))
        run(qk_wave(1, 0))
        run(v_wave(0))
        fillers = [qk_wave(0, 1), qk_wave(1, 1), v_wave(1)]
        weave(attention_pair(0, scps_a, scps_a, "scp", 2), fillers)
        weave(attention_pair(1, scps_a, scps_a, "scp", 2), fillers)
        for fg in fillers:
            run(fg)
        phase1.close()

        # phase 2: pairs 2-3 woven with output projections
        scps = ctx.enter_context(tc.tile_pool(name="scps", bufs=1,
                                              space="PSUM"))
        tpps = ctx.enter_context(tc.tile_pool(name="tpps", bufs=1,
                                              space="PSUM"))
        fo = ctx.enter_context(tc.tile_pool(name="fo", bufs=1, space="PSUM"))
        ag_chunk(0)
        ag_chunk(1)
        fillers = [out_proj(0, fo, "fo"), out_proj(1, fo, "fo")]
        weave(attention_pair(2, scps, tpps, "tp", 4), fillers)
        ag_chunk(2)
        fillers += [out_proj(2, fo, "fo"), out_proj(3, fo, "fo")]
        weave(attention_pair(3, scps, tpps, "tp", 4), fillers)
        ag_chunk(3)
        run(out_proj(4, scps, "scp"))
        run(out_proj(5, scps, "scp"))
        run(out_proj(6, scps, "scp"))
        run(out_proj(7, scps, "scp"))

    nc.compile()
    return nc


def _get_nc(causal: bool, tmin: int):
    key = (causal, tmin)
    if key not in _NC_CACHE:
        _NC_CACHE[key] = _build(causal, tmin)
    return _NC_CACHE[key]


def kernel(x, pe, content_mask, padding_mask, Wqkv, bqkv, Wo, bo):
    global LAST_RESULT
    x = np.asarray(x, dtype=np.float32)
    pe = np.asarray(pe, dtype=np.float32)
    content_mask = np.asarray(content_mask, dtype=bool)
    padding_mask = np.asarray(padding_mask, dtype=bool)
    Wqkv = np.asarray(Wqkv, dtype=np.float32)
    bqkv = np.asarray(bqkv, dtype=np.float32)
    Wo = np.asarray(Wo, dtype=np.float32)
    bo = np.asarray(bo, dtype=np.float32)
    assert x.shape == (S, B, C) and Wqkv.shape == (3 * C, C)

    causal_2d = np.triu(np.ones((S, S), dtype=bool), 1)
    causal = np.array_equal(content_mask,
                            np.broadcast_to(causal_2d[:, :, None], (S, S, B)))
    if causal:
        first_pad = S
        for b in range(B):
            col = padding_mask[:, b]
            if col.any():
                first_pad = min(first_pad, int(np.argmax(col)))
        tmin = first_pad // P
    else:
        tmin = 0

    nc = _get_nc(causal, tmin)

    in_maps = []
    for core in range(N_CORES):
        b, hg = core // 2, core % 2
        xpe_sel = slice(None)
        m = {
            "xT": np.ascontiguousarray(x[:, b, :].T.astype(BF16)),
            "peT": np.ascontiguousarray(pe[:, b, :].T.astype(BF16)),
        }
        rows = np.concatenate([
            np.arange(F * hg, F * (hg + 1)),
            np.arange(C + F * hg, C + F * (hg + 1)),
            np.arange(2 * C + F * hg, 2 * C + F * (hg + 1)),
        ])
        m["wqkvT"] = np.ascontiguousarray(Wqkv[rows, :].T.astype(BF16))
        m["woT"] = np.ascontiguousarray(
        Wo[(C // 2) * hg:(C // 2) * (hg + 1), :].T.astype(BF16))
        bq = bqkv[F * hg:F * (hg + 1)]
        bk = bqkv[C + F * hg:C + F * (hg + 1)]
        bv = bqkv[2 * C + F * hg:2 * C + F * (hg + 1)]
        m["bq_row"] = bq.reshape(1, F).astype(BF16)
        m["bk_row"] = bk.reshape(1, F).astype(BF16)
        m["bv_row"] = bv.reshape(1, F).astype(BF16)
        m["bo_row"] = bo[(C // 2) * hg:(C // 2) * (hg + 1)].reshape(1, -1).astype(BF16)
        if causal:
            keep = (~padding_mask[:, b]).astype(np.float32)  # [S]
            m["pad01"] = np.ascontiguousarray(keep.reshape(NT, P).T)
            m["pad_colb"] = np.ascontiguousarray(np.broadcast_to(
                m["pad01"][:, :, None], (P, NT, HG)).astype(BF16))
        else:
            keep2d = ~(content_mask[:, :, b] | padding_mask[None, :, b])  # [s, t]
            mT = keep2d.T.astype(BF16)  # [t, s]
            m["maskT"] = np.ascontiguousarray(mT.reshape(NT, P, S).transpose(1, 0, 2))
        in_maps.append(m)

    trace = bool(os.environ.get("BASS_KERNEL_TRACE"))
    res = run_bass_kernel_spmd(nc, in_maps, core_ids=list(range(N_CORES)),
                               trace=trace)
    LAST_RESULT = res

    out = np.empty((S, B, C), dtype=np.float32)
    for core in range(N_CORES):
        b, hg = core // 2, core % 2
        out[:, b, (C // 2) * hg:(C // 2) * (hg + 1)] = res.results[core]["out"]
    return out


# revision 27
# speedup vs baseline: 1.0065x; 1.0065x over previous
"""Trainium2 Bass kernel for CompressedCausalAttention.

Reference computation (S=1024, B=4, C=1024, H=16, CC=64):
    qkv = (x + pe) @ Wqkv.T + bqkv
    q, k, v = split(qkv); reshape to [s, b, H, CC]
    qk = einsum('sbhc,tbhc->stbh', q, k) / sqrt(CC)
    mask = content_mask[:,:,:,None] | padding_mask[None,:,:,None]
    p = softmax(where(mask, -inf, qk), axis=1)
    out = einsum('stbh,tbhc->sbhc', p, v).reshape(s,b,c) @ Wo.T + bo

Sharding: 8 cores = (batch b in 0..3) x (head-group hg in 0..1, 8 heads each).
Each core projects q/k/v for its 8 heads of its batch, computes attention with
scores in transposed [t, s] layout (softmax row-sum comes free from an
appended ones-column in V; no max-subtraction needed at these magnitudes),
pair-AllGathers the per-head attention outputs within a batch, and computes
the output projection for its half of the output channels (host-sliced Wo
keeps the SPMD program identical on all cores).
"""

import os
import sys
import types

import ml_dtypes
import numpy as np

_SO_PATH = "/opt/axon/libaxon_pjrt.so"


def _install_ntff_shim():
    """Make `antenv.axon_hooks` importable so trace=True works under axon."""
    try:
        from antenv.axon_hooks import set_axon_ntff_profile_hook  # noqa: F401

        return
    except ImportError:
        pass
    try:
        import antenv
        import trn_agent_boot.trn_boot as tb
    except ImportError:
        return
    mod = types.ModuleType("antenv.axon_hooks")
    _hook = [None]
    mod.set_axon_ntff_profile_hook = lambda h: _hook.__setitem__(0, h)
    mod.get_axon_ntff_profile_hook = lambda: _hook[0]
    sys.modules["antenv.axon_hooks"] = mod
    antenv.axon_hooks = mod
    if os.path.exists(_SO_PATH):
        mod.set_axon_ntff_profile_hook(tb._ntff_profile_via_ctypes(_SO_PATH))


_install_ntff_shim()

import concourse.bass as bass  # noqa: E402
import concourse.tile as tile  # noqa: E402
from concourse import bacc, mybir  # noqa: E402
from concourse.bass_utils import run_bass_kernel_spmd  # noqa: E402

S = 1024
B = 4
C = 1024
H = 16
CC = 64
HG = 8  # heads per core
F = HG * CC  # 512 features per core for each of q/k/v
P = 128
NQ = S // P  # 8 query tiles
NT = S // P  # 8 key tiles
KT = C // P  # 8 contraction tiles
TEMP = 1.0 / 8.0

DTB = mybir.dt.bfloat16
DTF = mybir.dt.float32
BF16 = ml_dtypes.bfloat16

N_CORES = 8
PAIRS = [[2 * b, 2 * b + 1] for b in range(B)]

_NC_CACHE = {}
WARMUP_MMS = 120
LAST_RESULT = None  # BassKernelResults of the most recent run (for profiling)


def _build(causal: bool, tmin: int):
    """Build the SPMD program. `causal`: triangular block skipping + tril
    mask on diagonal score blocks; padding is folded into V (padded rows and
    the denominator column are zeroed). Otherwise a full [t, s] 0/1 mask
    input is applied per score block."""
    nc = bacc.Bacc("TRN2", target_bir_lowering=False, debug=False,
                   num_devices=N_CORES)

    xT = nc.dram_tensor("xT", [C, S], DTB, kind="ExternalInput")
    peT = nc.dram_tensor("peT", [C, S], DTB, kind="ExternalInput")
    wqkvT = nc.dram_tensor("wqkvT", [C, 3 * F], DTB, kind="ExternalInput")
    woT = nc.dram_tensor("woT", [C, C // 2], DTB, kind="ExternalInput")
    bq_row = nc.dram_tensor("bq_row", [1, F], DTB, kind="ExternalInput")
    bk_row = nc.dram_tensor("bk_row", [1, F], DTB, kind="ExternalInput")
    bv_row = nc.dram_tensor("bv_row", [1, F], DTB, kind="ExternalInput")
    bo_row = nc.dram_tensor("bo_row", [1, C // 2], DTB, kind="ExternalInput")
    if causal:
        pad01 = nc.dram_tensor("pad01", [P, NT], DTF, kind="ExternalInput")
        pad_colb = nc.dram_tensor("pad_colb", [P, NT, HG], DTB,
                                  kind="ExternalInput")
        maskT = None
    else:
        pad01 = None
        maskT = nc.dram_tensor("maskT", [P, NT, S], DTB, kind="ExternalInput")
    out_h = nc.dram_tensor("out", [S, C // 2], DTF, kind="ExternalOutput")

    # chunked pair-AllGather buffers (token quarters)
    NAG = 4
    AGW = S // NAG
    ag_in = [nc.dram_tensor(f"ag_in{i}", [F, AGW], DTB) for i in range(NAG)]
    ag_out = [nc.dram_tensor(f"ag_out{i}", [2 * F, AGW], DTB)
              for i in range(NAG)]

    tril_np = np.triu(np.ones((P, P))).astype(BF16)  # keep t <= s
    tril_dram = nc.inline_tensor(tril_np, name="tril_const")
    ident_dram = nc.inline_tensor(np.eye(P).astype(BF16), name="ident_const")

    cpy = mybir.ActivationFunctionType.Copy
    from contextlib import ExitStack

    with tile.TileContext(nc) as tc, ExitStack() as ctx:
        cpool = ctx.enter_context(tc.tile_pool(name="const", bufs=1))
        pp = ctx.enter_context(tc.tile_pool(name="persist", bufs=1))

        xpe = pp.tile([P, KT, S], DTB)
        wq_b = pp.tile([P, KT, 3 * F], DTB)
        wo_b = pp.tile([P, KT, C // 2], DTB)
        q_t = pp.tile([P, F // P, S], DTB)
        k_t = pp.tile([P, F // P, S], DTB)
        v_t = pp.tile([P, NT, HG * (CC + 1)], DTB)
        attn_l = pp.tile([P, F // P, S], DTB)     # attn^T local [hc, tok]
        attn_f = pp.tile([P, KT, S], DTB)         # attn^T both head-groups

        tril = cpool.tile([P, P], DTB)
        nc.sync.dma_start(tril[:], tril_dram[:])
        ident = cpool.tile([P, P], DTB)
        nc.sync.dma_start(ident[:], ident_dram[:])
        ones_r = cpool.tile([1, P], DTB)
        nc.vector.memset(ones_r[:], 1.0)
        ones_half = cpool.tile([1, S // 2], DTB)
        nc.vector.memset(ones_half[:], 1.0)
        warm_rhs = cpool.tile([P, 256], DTB)
        nc.vector.memset(warm_rhs[:], 0.0)
        bq_t = cpool.tile([1, F], DTB)
        nc.sync.dma_start(bq_t[:], bq_row[:])
        bk_t = cpool.tile([1, F], DTB)
        nc.sync.dma_start(bk_t[:], bk_row[:])
        bv_t = cpool.tile([1, F], DTB)
        nc.sync.dma_start(bv_t[:], bv_row[:])
        bo_t = cpool.tile([1, C // 2], DTB)
        nc.sync.dma_start(bo_t[:], bo_row[:])
        if causal:
            pad_t = cpool.tile([P, NT], DTF)
            nc.sync.dma_start(pad_t[:], pad01[:])
            pad_cb = cpool.tile([P, NT, HG], DTB)
            nc.sync.dma_start(pad_cb[:], pad_colb[:])
        else:
            m_t = pp.tile([P, NT, S], DTB)
            for tj in range(NT):
                nc.sync.dma_start(m_t[:, tj, :], maskT[:, tj, :])

        # ---- PE warmup: dummy matmuls fill the input-DMA window so the
        # HAM clock gate is released before the projections start ----
        with tc.tile_pool(name="warmps", bufs=1, space="PSUM") as warmps:
            wps = warmps.tile([P, 256], DTF)
            for _ in range(WARMUP_MMS):
                nc.tensor.matmul(wps[:], ident[:], warm_rhs[:])

        # ---- load + convert inputs to bf16 (x/pe and qkv weights) ----
        stage = ctx.enter_context(tc.tile_pool(name="stage", bufs=4))
        for kt in range(KT):
            nc.sync.dma_start(xpe[:, kt, :], xT[P * kt:P * (kt + 1), :])
            ps_ = stage.tile([P, S], DTB, tag="ps")
            nc.gpsimd.dma_start(ps_[:], peT[P * kt:P * (kt + 1), :])
            nc.scalar.dma_start(wq_b[:, kt, :], wqkvT[P * kt:P * (kt + 1), :])
            nc.vector.tensor_add(xpe[:, kt, :], xpe[:, kt, :], ps_[:])
        for kt in range(KT):
            nc.gpsimd.dma_start(wo_b[:, kt, :], woT[P * kt:P * (kt + 1), :])

        # ---- projections, in kt-outer "waves" so the matmuls stream right
        # behind the weight DMAs. Emission of attention score chunks is WOVEN
        # with slices of projection / output-projection matmuls: the PE
        # instruction stream is strictly in-order, so each score->exp
        # ping-pong stall is filled with independent dense matmul work ----
        v3 = v_t[:].rearrange("p n (h c) -> p n h c", c=CC + 1)
        if causal:
            nc.vector.tensor_copy(v3[:, :, :, CC], pad_cb[:])
        else:
            nc.vector.memset(v3[:, :, :, CC], 1.0)

        ep = ctx.enter_context(tc.tile_pool(name="ep", bufs=2))
        ptp = ctx.enter_context(tc.tile_pool(name="ptp", bufs=3))
        osbp = ctx.enter_context(tc.tile_pool(name="osb", bufs=3))

        ops_pool = ctx.enter_context(tc.tile_pool(name="ops", bufs=1,
                                                  space="PSUM"))
        phase1 = ExitStack()
        projps = phase1.enter_context(tc.tile_pool(name="projps", bufs=1,
                                                   space="PSUM"))
        scps_a = phase1.enter_context(tc.tile_pool(name="scpsa", bufs=1,
                                                   space="PSUM"))

        def qk_wave(which, nh):
            # generator: project 4 feature tiles of q (which=0) / k (which=1)
            tiles = [projps.tile([P, S // 2], DTF, tag=f"pj{i}",
                                 name=f"psq{i}") for i in range(4)]
            for kt in range(KT):
                for ft in range(4):
                    nc.tensor.matmul(
                        tiles[ft][:],
                        wq_b[:, kt, F * which + P * ft:F * which + P * (ft + 1)],
                        xpe[:, kt, (S // 2) * nh:(S // 2) * (nh + 1)],
                        start=(kt == 0), stop=False,
                    )
                yield
            bias_t = bq_t if which == 0 else bk_t
            dst = q_t if which == 0 else k_t
            for ft in range(4):
                nc.tensor.matmul(tiles[ft][:], bias_t[:, P * ft:P * (ft + 1)],
                                 ones_half[:, :], start=False, stop=True)
                nc.vector.tensor_copy(
                    dst[:, ft, (S // 2) * nh:(S // 2) * (nh + 1)], tiles[ft][:])

        def v_wave(half):
            tiles = [projps.tile([P, F], DTF, tag=f"pj{i}", name=f"psv{i}")
                     for i in range(4)]
            for kt in range(KT):
                for i in range(4):
                    tt = 4 * half + i
                    nc.tensor.matmul(
                        tiles[i][:], xpe[:, kt, P * tt:P * (tt + 1)],
                        wq_b[:, kt, 2 * F:3 * F],
                        start=(kt == 0), stop=False,
                    )
                yield
            for i in range(4):
                tt = 4 * half + i
                nc.tensor.matmul(tiles[i][:], ones_r[:, :], bv_t[:, :],
                                 start=False, stop=True)
                if causal:
                    nc.vector.tensor_scalar_mul(
                        v3[:, tt, :, 0:CC],
                        tiles[i][:].rearrange("p (h c) -> p h c", c=CC),
                        pad_t[:, tt:tt + 1],
                    )
                else:
                    nc.vector.tensor_copy(
                        v3[:, tt, :, 0:CC],
                        tiles[i][:].rearrange("p (h c) -> p h c", c=CC),
                    )

        def attention_pair(qp, scpool, tppool, tptag, ctj):
            # generator: scores/exp chunks (ctj key tiles each) yield between
            # chunks so filler matmuls can be woven into the PE stream
            q0, q1 = 2 * qp, 2 * qp + 1
            n_t = q1 + 1 if causal else NT
            pts = []
            for hp in range(HG // 2):
                ft = hp
                pt = ptp.tile([P, 2, NT, 2 * P], DTB, tag="pt", name="pt")
                pts.append(pt)
                for c0 in range(0, n_t, ctj):
                    cn = min(ctj, n_t - c0)
                    scp = scpool.tile([P, 2, ctj, 2 * P], DTF, tag="scp",
                                      name="scp")
                    for tj in range(c0, c0 + cn):
                        nc.tensor.matmul(
                            scp[:, 0, tj - c0, :],
                            k_t[0:CC, ft, P * tj:P * (tj + 1)],
                            q_t[0:CC, ft, 2 * P * qp:2 * P * (qp + 1)],
                        )
                        nc.tensor.matmul(
                            scp[:, 1, tj - c0, :],
                            k_t[CC:P, ft, P * tj:P * (tj + 1)],
                            q_t[CC:P, ft, 2 * P * qp:2 * P * (qp + 1)],
                        )
                    nc.scalar.activation(
                        pt[:, :, c0:c0 + cn, :], scp[:, :, 0:cn, :],
                        mybir.ActivationFunctionType.Exp, scale=TEMP)
                    yield
            for iq, qi in enumerate((q0, q1)):
                nt_i = qi + 1 if causal else NT
                out_ab = [ops_pool.tile([P, HG // 2, CC + 1], DTF,
                                        tag=f"out{x}", name=f"out_ab{x}")
                          for x in range(2)]
                for hp in range(HG // 2):
                    pt = pts[hp]
                    if causal:
                        nc.vector.tensor_mul(
                            pt[:, :, qi, P * iq:P * (iq + 1)],
                            pt[:, :, qi, P * iq:P * (iq + 1)], tril2[:])
                    else:
                        for tj in range(nt_i):
                            nc.vector.tensor_mul(
                                pt[:, :, tj, P * iq:P * (iq + 1)],
                                pt[:, :, tj, P * iq:P * (iq + 1)],
                                m2_t[:, :, tj, P * qi:P * (qi + 1)])
                    for x, h in ((0, 2 * hp), (1, 2 * hp + 1)):
                        for tj in range(nt_i):
                            nc.tensor.matmul(
                                out_ab[h // 4][:, h % 4, :],
                                pt[:, x, tj, P * iq:P * (iq + 1)],
                                v_t[:, tj, (CC + 1) * h:(CC + 1) * (h + 1)],
                                start=(tj == 0), stop=(tj == nt_i - 1),
                            )
                    yield
                # normalization epilogue for this query tile
                rec = ep.tile([P, HG], DTF, tag="rec", name="rec")
                for x in range(2):
                    nc.vector.reciprocal(rec[:, 4 * x:4 * (x + 1)],
                                         out_ab[x][:, :, CC])
                attn_s = ep.tile([P, F], DTB, tag="attn_s", name="attn_s")
                for h in range(HG):
                    nc.vector.tensor_scalar_mul(
                        attn_s[:, CC * h:CC * (h + 1)],
                        out_ab[h // 4][:, h % 4, 0:CC],
                        rec[:, h:h + 1],
                    )
                tp = tppool.tile([P, HG // 2, P], DTB, tag=tptag, name="tp")
                for hp in range(HG // 2):
                    nc.tensor.transpose(tp[:, hp, :],
                                        attn_s[:, P * hp:P * (hp + 1)],
                                        ident[:])
                    nc.vector.tensor_copy(attn_l[:, hp, P * qi:P * (qi + 1)],
                                          tp[:, hp, :])
                yield

        def ag_chunk(i):
            for ft in range(F // P):
                nc.sync.dma_start(
                    ag_in[i][P * ft:P * (ft + 1), :],
                    attn_l[:, ft, AGW * i:AGW * (i + 1)])
            nc.gpsimd.collective_compute(
                "AllGather", mybir.AluOpType.bypass, replica_groups=PAIRS,
                ins=[ag_in[i][:]], outs=[ag_out[i][:]],
            )
            for kc in range(KT):
                nc.gpsimd.dma_start(
                    attn_f[:, kc, AGW * i:AGW * (i + 1)],
                    ag_out[i][P * kc:P * (kc + 1), :])

        def out_proj(mt, pool, tag):
            psf = pool.tile([P, C // 2], DTF, tag=tag, name="psf")
            for kc in range(KT):
                nc.tensor.matmul(
                    psf[:], attn_f[:, kc, P * mt:P * (mt + 1)],
                    wo_b[:, kc, :],
                    start=(kc == 0), stop=False,
                )
                if kc == 3:
                    yield
            nc.tensor.matmul(psf[:], ones_r[:, :], bo_t[:, :],
                             start=False, stop=True)
            osb = osbp.tile([P, C // 2], DTF, tag="osb", name="osb")
            nc.scalar.copy(osb[:], psf[:])
            nc.sync.dma_start(out_h[P * mt:P * (mt + 1), :], osb[:])
            yield

        def weave(main_gen, fillers):
            """Run main_gen; after each of its yields, advance the current
            filler generator by one step."""
            for _ in main_gen:
                while fillers:
                    try:
                        next(fillers[0])
                        break
                    except StopIteration:
                        fillers.pop(0)
            for fg in fillers:
                for _ in fg:
                    pass
            fillers.clear()

        def run(gen):
            for _ in gen:
                pass

        # doubled tril (for masking both heads of a pair in one op)
        tril2 = cpool.tile([P, 2, P], DTB)
        for x in range(2):
            nc.vector.tensor_copy(tril2[:, x, :], tril[:])
        if not causal:
            m2_t = pp.tile([P, 2, NT, S], DTB)
            for x in range(2):
                nc.vector.tensor_copy(m2_t[:, x, :, :], m_t[:])

        # phase 1: q/k/v token-half 0 dense; then pairs 0-1 woven with the
        # remaining projection waves
        run(qk_wave(0, 0))
        run(qk_wave(1, 0))
        run(v_wave(0))
        fillers = [qk_wave(0, 1), qk_wave(1, 1), v_wave(1)]
        weave(attention_pair(0, scps_a, scps_a, "scp", 2), fillers)
        ag_chunk(0)
        weave(attention_pair(1, scps_a, scps_a, "scp", 2), fillers)
        ag_chunk(1)
        for fg in fillers:
            run(fg)
        phase1.close()

        # phase 2: pairs 2-3 woven with output projections
        scps = ctx.enter_context(tc.tile_pool(name="scps", bufs=1,
                                              space="PSUM"))
        tpps = ctx.enter_context(tc.tile_pool(name="tpps", bufs=1,
                                              space="PSUM"))
        fo = ctx.enter_context(tc.tile_pool(name="fo", bufs=1, space="PSUM"))
        fillers = [out_proj(0, fo, "fo"), out_proj(1, fo, "fo")]
        weave(attention_pair(2, scps, tpps, "tp", 4), fillers)
        ag_chunk(2)
        fillers += [out_proj(2, fo, "fo"), out_proj(3, fo, "fo")]
        weave(attention_pair(3, scps, tpps, "tp", 4), fillers)
        ag_chunk(3)
        run(out_proj(4, scps, "scp"))
        run(out_proj(5, scps, "scp"))
        run(out_proj(6, scps, "scp"))
        run(out_proj(7, scps, "scp"))

    nc.compile()
    return nc


def _get_nc(causal: bool, tmin: int):
    key = (causal, tmin)
    if key not in _NC_CACHE:
        _NC_CACHE[key] = _build(causal, tmin)
    return _NC_CACHE[key]


def kernel(x, pe, content_mask, padding_mask, Wqkv, bqkv, Wo, bo):
    global LAST_RESULT
    x = np.asarray(x, dtype=np.float32)
    pe = np.asarray(pe, dtype=np.float32)
    content_mask = np.asarray(content_mask, dtype=bool)
    padding_mask = np.asarray(padding_mask, dtype=bool)
    Wqkv = np.asarray(Wqkv, dtype=np.float32)
    bqkv = np.asarray(bqkv, dtype=np.float32)
    Wo = np.asarray(Wo, dtype=np.float32)
    bo = np.asarray(bo, dtype=np.float32)
    assert x.shape == (S, B, C) and Wqkv.shape == (3 * C, C)

    causal_2d = np.triu(np.ones((S, S), dtype=bool), 1)
    causal = np.array_equal(content_mask,
                            np.broadcast_to(causal_2d[:, :, None], (S, S, B)))
    if causal:
        first_pad = S
        for b in range(B):
            col = padding_mask[:, b]
            if col.any():
                first_pad = min(first_pad, int(np.argmax(col)))
        tmin = first_pad // P
    else:
        tmin = 0

    nc = _get_nc(causal, tmin)

    in_maps = []
    for core in range(N_CORES):
        b, hg = core // 2, core % 2
        xpe_sel = slice(None)
        m = {
            "xT": np.ascontiguousarray(x[:, b, :].T.astype(BF16)),
            "peT": np.ascontiguousarray(pe[:, b, :].T.astype(BF16)),
        }
        rows = np.concatenate([
            np.arange(F * hg, F * (hg + 1)),
            np.arange(C + F * hg, C + F * (hg + 1)),
            np.arange(2 * C + F * hg, 2 * C + F * (hg + 1)),
        ])
        m["wqkvT"] = np.ascontiguousarray(Wqkv[rows, :].T.astype(BF16))
        m["woT"] = np.ascontiguousarray(
        Wo[(C // 2) * hg:(C // 2) * (hg + 1), :].T.astype(BF16))
        bq = bqkv[F * hg:F * (hg + 1)]
        bk = bqkv[C + F * hg:C + F * (hg + 1)]
        bv = bqkv[2 * C + F * hg:2 * C + F * (hg + 1)]
        m["bq_row"] = bq.reshape(1, F).astype(BF16)
        m["bk_row"] = bk.reshape(1, F).astype(BF16)
        m["bv_row"] = bv.reshape(1, F).astype(BF16)
        m["bo_row"] = bo[(C // 2) * hg:(C // 2) * (hg + 1)].reshape(1, -1).astype(BF16)
        if causal:
            keep = (~padding_mask[:, b]).astype(np.float32)  # [S]
            m["pad01"] = np.ascontiguousarray(keep.reshape(NT, P).T)
            m["pad_colb"] = np.ascontiguousarray(np.broadcast_to(
                m["pad01"][:, :, None], (P, NT, HG)).astype(BF16))
        else:
            keep2d = ~(content_mask[:, :, b] | padding_mask[None, :, b])  # [s, t]
            mT = keep2d.T.astype(BF16)  # [t, s]
            m["maskT"] = np.ascontiguousarray(mT.reshape(NT, P, S).transpose(1, 0, 2))
        in_maps.append(m)

    trace = bool(os.environ.get("BASS_KERNEL_TRACE"))
    res = run_bass_kernel_spmd(nc, in_maps, core_ids=list(range(N_CORES)),
                               trace=trace)
    LAST_RESULT = res

    out = np.empty((S, B, C), dtype=np.float32)
    for core in range(N_CORES):
        b, hg = core // 2, core % 2
        out[:, b, (C // 2) * hg:(C // 2) * (hg + 1)] = res.results[core]["out"]
    return out


# revision 29
# speedup vs baseline: 1.1025x; 1.0953x over previous
"""Trainium2 Bass kernel for CompressedCausalAttention.

Reference computation (S=1024, B=4, C=1024, H=16, CC=64):
    qkv = (x + pe) @ Wqkv.T + bqkv
    q, k, v = split(qkv); reshape to [s, b, H, CC]
    qk = einsum('sbhc,tbhc->stbh', q, k) / sqrt(CC)
    mask = content_mask[:,:,:,None] | padding_mask[None,:,:,None]
    p = softmax(where(mask, -inf, qk), axis=1)
    out = einsum('stbh,tbhc->sbhc', p, v).reshape(s,b,c) @ Wo.T + bo

Sharding: 8 cores = (batch b in 0..3) x (head-group hg in 0..1, 8 heads each).
Each core projects q/k/v for its 8 heads of its batch, computes attention with
scores in transposed [t, s] layout (softmax row-sum comes free from an
appended ones-column in V; no max-subtraction needed at these magnitudes),
pair-AllGathers the per-head attention outputs within a batch, and computes
the output projection for its half of the output channels (host-sliced Wo
keeps the SPMD program identical on all cores).
"""

import os
import sys
import types

import ml_dtypes
import numpy as np

_SO_PATH = "/opt/axon/libaxon_pjrt.so"


def _install_ntff_shim():
    """Make `antenv.axon_hooks` importable so trace=True works under axon."""
    try:
        from antenv.axon_hooks import set_axon_ntff_profile_hook  # noqa: F401

        return
    except ImportError:
        pass
    try:
        import antenv
        import trn_agent_boot.trn_boot as tb
    except ImportError:
        return
    mod = types.ModuleType("antenv.axon_hooks")
    _hook = [None]
    mod.set_axon_ntff_profile_hook = lambda h: _hook.__setitem__(0, h)
    mod.get_axon_ntff_profile_hook = lambda: _hook[0]
    sys.modules["antenv.axon_hooks"] = mod
    antenv.axon_hooks = mod
    if os.path.exists(_SO_PATH):
        mod.set_axon_ntff_profile_hook(tb._ntff_profile_via_ctypes(_SO_PATH))


_install_ntff_shim()

import concourse.bass as bass  # noqa: E402
import concourse.tile as tile  # noqa: E402
from concourse import bacc, mybir  # noqa: E402
from concourse.bass_utils import run_bass_kernel_spmd  # noqa: E402

S = 1024
B = 4
C = 1024
H = 16
CC = 64
HG = 8  # heads per core
F = HG * CC  # 512 features per core for each of q/k/v
P = 128
NQ = S // P  # 8 query tiles
NT = S // P  # 8 key tiles
KT = C // P  # 8 contraction tiles
TEMP = 1.0 / 8.0

DTB = mybir.dt.bfloat16
DTF = mybir.dt.float32
BF16 = ml_dtypes.bfloat16

N_CORES = 8
PAIRS = [[2 * b, 2 * b + 1] for b in range(B)]

_NC_CACHE = {}
WARMUP_MMS = 120
LAST_RESULT = None  # BassKernelResults of the most recent run (for profiling)


def _build(causal: bool, tmin: int):
    """Build the SPMD program. `causal`: triangular block skipping + tril
    mask on diagonal score blocks; padding is folded into V (padded rows and
    the denominator column are zeroed). Otherwise a full [t, s] 0/1 mask
    input is applied per score block."""
    nc = bacc.Bacc("TRN2", target_bir_lowering=False, debug=False,
                   num_devices=N_CORES)

    xT = nc.dram_tensor("xT", [C, S], DTB, kind="ExternalInput")
    peT = nc.dram_tensor("peT", [C, S], DTB, kind="ExternalInput")
    wqkvT = nc.dram_tensor("wqkvT", [C, 3 * F], DTB, kind="ExternalInput")
    woT = nc.dram_tensor("woT", [C, C // 2], DTB, kind="ExternalInput")
    bq_row = nc.dram_tensor("bq_row", [1, F], DTB, kind="ExternalInput")
    bk_row = nc.dram_tensor("bk_row", [1, F], DTB, kind="ExternalInput")
    bv_row = nc.dram_tensor("bv_row", [1, F], DTB, kind="ExternalInput")
    bo_row = nc.dram_tensor("bo_row", [1, C // 2], DTB, kind="ExternalInput")
    if causal:
        pad01 = nc.dram_tensor("pad01", [P, NT], DTF, kind="ExternalInput")
        pad_colb = nc.dram_tensor("pad_colb", [P, NT, HG], DTB,
                                  kind="ExternalInput")
        maskT = None
    else:
        pad01 = None
        maskT = nc.dram_tensor("maskT", [P, NT, S], DTB, kind="ExternalInput")
    out_h = nc.dram_tensor("out", [S, C // 2], DTF, kind="ExternalOutput")

    # chunked pair-AllGather buffers (token quarters)
    NAG = 4
    AGW = S // NAG
    ag_in = [nc.dram_tensor(f"ag_in{i}", [F, AGW], DTB) for i in range(NAG)]
    ag_out = [nc.dram_tensor(f"ag_out{i}", [2 * F, AGW], DTB)
              for i in range(NAG)]

    tril_np = np.triu(np.ones((P, P))).astype(BF16)  # keep t <= s
    tril_dram = nc.inline_tensor(tril_np, name="tril_const")
    ident_dram = nc.inline_tensor(np.eye(P).astype(BF16), name="ident_const")

    cpy = mybir.ActivationFunctionType.Copy
    from contextlib import ExitStack

    with tile.TileContext(nc) as tc, ExitStack() as ctx:
        cpool = ctx.enter_context(tc.tile_pool(name="const", bufs=1))
        pp = ctx.enter_context(tc.tile_pool(name="persist", bufs=1))

        xpe = pp.tile([P, KT, S], DTB)
        wq_b = pp.tile([P, KT, 3 * F], DTB)
        wo_b = pp.tile([P, KT, C // 2], DTB)
        q_t = pp.tile([P, F // P, S], DTB)
        k_t = pp.tile([P, F // P, S], DTB)
        v_t = pp.tile([P, NT, HG * (CC + 1)], DTB)
        attn_l = pp.tile([P, F // P, S], DTB)     # attn^T local [hc, tok]
        attn_f = pp.tile([P, KT, S], DTB)         # attn^T both head-groups

        tril = cpool.tile([P, P], DTB)
        nc.sync.dma_start(tril[:], tril_dram[:])
        ident = cpool.tile([P, P], DTB)
        nc.sync.dma_start(ident[:], ident_dram[:])
        ones_r = cpool.tile([1, P], DTB)
        nc.vector.memset(ones_r[:], 1.0)
        ones_half = cpool.tile([1, S // 2], DTB)
        nc.vector.memset(ones_half[:], 1.0)
        warm_rhs = cpool.tile([P, 256], DTB)
        nc.vector.memset(warm_rhs[:], 0.0)
        bq_t = cpool.tile([1, F], DTB)
        nc.sync.dma_start(bq_t[:], bq_row[:])
        bk_t = cpool.tile([1, F], DTB)
        nc.sync.dma_start(bk_t[:], bk_row[:])
        bv_t = cpool.tile([1, F], DTB)
        nc.sync.dma_start(bv_t[:], bv_row[:])
        bo_t = cpool.tile([1, C // 2], DTB)
        nc.sync.dma_start(bo_t[:], bo_row[:])
        if causal:
            pad_t = cpool.tile([P, NT], DTF)
            nc.sync.dma_start(pad_t[:], pad01[:])
            pad_cb = cpool.tile([P, NT, HG], DTB)
            nc.sync.dma_start(pad_cb[:], pad_colb[:])
        else:
            m_t = pp.tile([P, NT, S], DTB)
            for tj in range(NT):
                nc.sync.dma_start(m_t[:, tj, :], maskT[:, tj, :])

        # ---- PE warmup: dummy matmuls fill the input-DMA window so the
        # HAM clock gate is released before the projections start ----
        with tc.tile_pool(name="warmps", bufs=1, space="PSUM") as warmps:
            wps = warmps.tile([P, 256], DTF)
            for _ in range(WARMUP_MMS):
                nc.tensor.matmul(wps[:], ident[:], warm_rhs[:])

        # ---- load + convert inputs to bf16 (x/pe and qkv weights) ----
        stage = ctx.enter_context(tc.tile_pool(name="stage", bufs=4))
        for kt in range(KT):
            nc.sync.dma_start(xpe[:, kt, :], xT[P * kt:P * (kt + 1), :])
            ps_ = stage.tile([P, S], DTB, tag="ps")
            nc.gpsimd.dma_start(ps_[:], peT[P * kt:P * (kt + 1), :])
            nc.scalar.dma_start(wq_b[:, kt, :], wqkvT[P * kt:P * (kt + 1), :])
            nc.vector.tensor_add(xpe[:, kt, :], xpe[:, kt, :], ps_[:])
        for kt in range(KT):
            nc.gpsimd.dma_start(wo_b[:, kt, :], woT[P * kt:P * (kt + 1), :])

        # ---- projections, in kt-outer "waves" so the matmuls stream right
        # behind the weight DMAs. Emission of attention score chunks is WOVEN
        # with slices of projection / output-projection matmuls: the PE
        # instruction stream is strictly in-order, so each score->exp
        # ping-pong stall is filled with independent dense matmul work ----
        v3 = v_t[:].rearrange("p n (h c) -> p n h c", c=CC + 1)
        if causal:
            nc.vector.tensor_copy(v3[:, :, :, CC], pad_cb[:])
        else:
            nc.vector.memset(v3[:, :, :, CC], 1.0)

        ep = ctx.enter_context(tc.tile_pool(name="ep", bufs=2))
        ptp = ctx.enter_context(tc.tile_pool(name="ptp", bufs=3))
        osbp = ctx.enter_context(tc.tile_pool(name="osb", bufs=3))

        phase1 = ExitStack()
        projps = phase1.enter_context(tc.tile_pool(name="projps", bufs=1,
                                                   space="PSUM"))

        def qk_wave(which, nh):
            # generator: project 4 feature tiles of q (which=0) / k (which=1)
            tiles = [projps.tile([P, S // 2], DTF, tag=f"pj{i}",
                                 name=f"psq{i}") for i in range(4)]
            for kt in range(KT):
                for ft in range(4):
                    nc.tensor.matmul(
                        tiles[ft][:],
                        wq_b[:, kt, F * which + P * ft:F * which + P * (ft + 1)],
                        xpe[:, kt, (S // 2) * nh:(S // 2) * (nh + 1)],
                        start=(kt == 0), stop=False,
                    )
                yield
            bias_t = bq_t if which == 0 else bk_t
            dst = q_t if which == 0 else k_t
            for ft in range(4):
                nc.tensor.matmul(tiles[ft][:], bias_t[:, P * ft:P * (ft + 1)],
                                 ones_half[:, :], start=False, stop=True)
                nc.vector.tensor_copy(
                    dst[:, ft, (S // 2) * nh:(S // 2) * (nh + 1)], tiles[ft][:])

        def v_wave(half):
            tiles = [projps.tile([P, F], DTF, tag=f"pj{i}", name=f"psv{i}")
                     for i in range(4)]
            for kt in range(KT):
                for i in range(4):
                    tt = 4 * half + i
                    nc.tensor.matmul(
                        tiles[i][:], xpe[:, kt, P * tt:P * (tt + 1)],
                        wq_b[:, kt, 2 * F:3 * F],
                        start=(kt == 0), stop=False,
                    )
                yield
            for i in range(4):
                tt = 4 * half + i
                nc.tensor.matmul(tiles[i][:], ones_r[:, :], bv_t[:, :],
                                 start=False, stop=True)
                if causal:
                    nc.vector.tensor_scalar_mul(
                        v3[:, tt, :, 0:CC],
                        tiles[i][:].rearrange("p (h c) -> p h c", c=CC),
                        pad_t[:, tt:tt + 1],
                    )
                else:
                    nc.vector.tensor_copy(
                        v3[:, tt, :, 0:CC],
                        tiles[i][:].rearrange("p (h c) -> p h c", c=CC),
                    )

        def attention_pair(qp, scpool, tppool, tptag, ctj):
            # generator: scores/exp chunks (ctj key tiles each) yield between
            # chunks so filler matmuls can be woven into the PE stream
            q0, q1 = 2 * qp, 2 * qp + 1
            n_t = q1 + 1 if causal else NT
            pts = []
            for hp in range(HG // 2):
                ft = hp
                pt = ptp.tile([P, 2, NT, 2 * P], DTB, tag="pt", name="pt")
                pts.append(pt)
                for c0 in range(0, n_t, ctj):
                    cn = min(ctj, n_t - c0)
                    scp = scpool.tile([P, 2, ctj, 2 * P], DTF, tag="scp",
                                      name="scp")
                    for tj in range(c0, c0 + cn):
                        nc.tensor.matmul(
                            scp[:, 0, tj - c0, :],
                            k_t[0:CC, ft, P * tj:P * (tj + 1)],
                            q_t[0:CC, ft, 2 * P * qp:2 * P * (qp + 1)],
                        )
                        nc.tensor.matmul(
                            scp[:, 1, tj - c0, :],
                            k_t[CC:P, ft, P * tj:P * (tj + 1)],
                            q_t[CC:P, ft, 2 * P * qp:2 * P * (qp + 1)],
                        )
                    nc.scalar.activation(
                        pt[:, :, c0:c0 + cn, :], scp[:, :, 0:cn, :],
                        mybir.ActivationFunctionType.Exp, scale=TEMP)
                    yield
            for iq, qi in enumerate((q0, q1)):
                nt_i = qi + 1 if causal else NT
                out_ab = [ops_pool.tile([P, HG // 2, CC + 1], DTF,
                                        tag=f"out{x}", name=f"out_ab{x}")
                          for x in range(2)]
                for hp in range(HG // 2):
                    pt = pts[hp]
                    if causal:
                        nc.vector.tensor_mul(
                            pt[:, :, qi, P * iq:P * (iq + 1)],
                            pt[:, :, qi, P * iq:P * (iq + 1)], tril2[:])
                    else:
                        for tj in range(nt_i):
                            nc.vector.tensor_mul(
                                pt[:, :, tj, P * iq:P * (iq + 1)],
                                pt[:, :, tj, P * iq:P * (iq + 1)],
                                m2_t[:, :, tj, P * qi:P * (qi + 1)])
                    for x, h in ((0, 2 * hp), (1, 2 * hp + 1)):
                        for tj in range(nt_i):
                            nc.tensor.matmul(
                                out_ab[h // 4][:, h % 4, :],
                                pt[:, x, tj, P * iq:P * (iq + 1)],
                                v_t[:, tj, (CC + 1) * h:(CC + 1) * (h + 1)],
                                start=(tj == 0), stop=(tj == nt_i - 1),
                            )
                    yield
                # normalization epilogue for this query tile
                rec = ep.tile([P, HG], DTF, tag="rec", name="rec")
                for x in range(2):
                    nc.vector.reciprocal(rec[:, 4 * x:4 * (x + 1)],
                                         out_ab[x][:, :, CC])
                attn_s = ep.tile([P, F], DTB, tag="attn_s", name="attn_s")
                for h in range(HG):
                    nc.vector.tensor_scalar_mul(
                        attn_s[:, CC * h:CC * (h + 1)],
                        out_ab[h // 4][:, h % 4, 0:CC],
                        rec[:, h:h + 1],
                    )
                tp = tppool.tile([P, HG // 2, P], DTB, tag=tptag, name="tp")
                for hp in range(HG // 2):
                    nc.tensor.transpose(tp[:, hp, :],
                                        attn_s[:, P * hp:P * (hp + 1)],
                                        ident[:])
                    nc.vector.tensor_copy(attn_l[:, hp, P * qi:P * (qi + 1)],
                                          tp[:, hp, :])
                yield

        def ag_chunk(i):
            for ft in range(F // P):
                nc.sync.dma_start(
                    ag_in[i][P * ft:P * (ft + 1), :],
                    attn_l[:, ft, AGW * i:AGW * (i + 1)])
            nc.gpsimd.collective_compute(
                "AllGather", mybir.AluOpType.bypass, replica_groups=PAIRS,
                ins=[ag_in[i][:]], outs=[ag_out[i][:]],
            )
            for kc in range(KT):
                nc.gpsimd.dma_start(
                    attn_f[:, kc, AGW * i:AGW * (i + 1)],
                    ag_out[i][P * kc:P * (kc + 1), :])

        def out_proj(mt, pool, tag):
            psf = pool.tile([P, C // 2], DTF, tag=tag, name="psf")
            for kc in range(KT):
                nc.tensor.matmul(
                    psf[:], attn_f[:, kc, P * mt:P * (mt + 1)],
                    wo_b[:, kc, :],
                    start=(kc == 0), stop=False,
                )
                if kc == 3:
                    yield
            nc.tensor.matmul(psf[:], ones_r[:, :], bo_t[:, :],
                             start=False, stop=True)
            osb = osbp.tile([P, C // 2], DTF, tag="osb", name="osb")
            nc.scalar.copy(osb[:], psf[:])
            nc.sync.dma_start(out_h[P * mt:P * (mt + 1), :], osb[:])
            yield

        def weave(main_gen, fillers):
            """Run main_gen; after each of its yields, advance the current
            filler generator by one step."""
            for _ in main_gen:
                while fillers:
                    try:
                        next(fillers[0])
                        break
                    except StopIteration:
                        fillers.pop(0)
            for fg in fillers:
                for _ in fg:
                    pass
            fillers.clear()

        def run(gen):
            for _ in gen:
                pass

        # doubled tril (for masking both heads of a pair in one op)
        tril2 = cpool.tile([P, 2, P], DTB)
        for x in range(2):
            nc.vector.tensor_copy(tril2[:, x, :], tril[:])
        if not causal:
            m2_t = pp.tile([P, 2, NT, S], DTB)
            for x in range(2):
                nc.vector.tensor_copy(m2_t[:, x, :, :], m_t[:])

        # phase 1: projections (dense kt-outer waves)
        run(qk_wave(0, 0))
        run(qk_wave(1, 0))
        run(v_wave(0))
        run(qk_wave(0, 1))
        run(qk_wave(1, 1))
        run(v_wave(1))
        phase1.close()

        # phase 2: attention pairs, AllGather chunks and output projections
        ops_pool = ctx.enter_context(tc.tile_pool(name="ops", bufs=1,
                                                  space="PSUM"))
        scps = ctx.enter_context(tc.tile_pool(name="scps", bufs=2,
                                              space="PSUM"))
        tpps = ctx.enter_context(tc.tile_pool(name="tpps", bufs=1,
                                              space="PSUM"))
        fo = ctx.enter_context(tc.tile_pool(name="fo", bufs=1, space="PSUM"))
        run(attention_pair(0, scps, tpps, "tp", 2))
        run(attention_pair(1, scps, tpps, "tp", 2))
        run(attention_pair(2, scps, tpps, "tp", 2))
        ag_chunk(0)
        ag_chunk(1)
        run(out_proj(0, fo, "fo"))
        run(out_proj(1, fo, "fo"))
        run(attention_pair(3, scps, tpps, "tp", 2))
        ag_chunk(2)
        run(out_proj(2, fo, "fo"))
        run(out_proj(3, fo, "fo"))
        ag_chunk(3)
        run(out_proj(4, scps, "scp"))
        run(out_proj(5, scps, "scp"))
        run(out_proj(6, scps, "scp"))
        run(out_proj(7, scps, "scp"))

    nc.compile()
    return nc


def _get_nc(causal: bool, tmin: int):
    key = (causal, tmin)
    if key not in _NC_CACHE:
        _NC_CACHE[key] = _build(causal, tmin)
    return _NC_CACHE[key]


def kernel(x, pe, content_mask, padding_mask, Wqkv, bqkv, Wo, bo):
    global LAST_RESULT
    x = np.asarray(x, dtype=np.float32)
    pe = np.asarray(pe, dtype=np.float32)
    content_mask = np.asarray(content_mask, dtype=bool)
    padding_mask = np.asarray(padding_mask, dtype=bool)
    Wqkv = np.asarray(Wqkv, dtype=np.float32)
    bqkv = np.asarray(bqkv, dtype=np.float32)
    Wo = np.asarray(Wo, dtype=np.float32)
    bo = np.asarray(bo, dtype=np.float32)
    assert x.shape == (S, B, C) and Wqkv.shape == (3 * C, C)

    causal_2d = np.triu(np.ones((S, S), dtype=bool), 1)
    causal = np.array_equal(content_mask,
                            np.broadcast_to(causal_2d[:, :, None], (S, S, B)))
    if causal:
        first_pad = S
        for b in range(B):
            col = padding_mask[:, b]
            if col.any():
                first_pad = min(first_pad, int(np.argmax(col)))
        tmin = first_pad // P
    else:
        tmin = 0

    nc = _get_nc(causal, tmin)

    in_maps = []
    for core in range(N_CORES):
        b, hg = core // 2, core % 2
        xpe_sel = slice(None)
        m = {
            "xT": np.ascontiguousarray(x[:, b, :].T.astype(BF16)),
            "peT": np.ascontiguousarray(pe[:, b, :].T.astype(BF16)),
        }
        rows = np.concatenate([
            np.arange(F * hg, F * (hg + 1)),
            np.arange(C + F * hg, C + F * (hg + 1)),
            np.arange(2 * C + F * hg, 2 * C + F * (hg + 1)),
        ])
        m["wqkvT"] = np.ascontiguousarray(Wqkv[rows, :].T.astype(BF16))
        m["woT"] = np.ascontiguousarray(
        Wo[(C // 2) * hg:(C // 2) * (hg + 1), :].T.astype(BF16))
        bq = bqkv[F * hg:F * (hg + 1)]
        bk = bqkv[C + F * hg:C + F * (hg + 1)]
        bv = bqkv[2 * C + F * hg:2 * C + F * (hg + 1)]
        m["bq_row"] = bq.reshape(1, F).astype(BF16)
        m["bk_row"] = bk.reshape(1, F).astype(BF16)
        m["bv_row"] = bv.reshape(1, F).astype(BF16)
        m["bo_row"] = bo[(C // 2) * hg:(C // 2) * (hg + 1)].reshape(1, -1).astype(BF16)
        if causal:
            keep = (~padding_mask[:, b]).astype(np.float32)  # [S]
            m["pad01"] = np.ascontiguousarray(keep.reshape(NT, P).T)
            m["pad_colb"] = np.ascontiguousarray(np.broadcast_to(
                m["pad01"][:, :, None], (P, NT, HG)).astype(BF16))
        else:
            keep2d = ~(content_mask[:, :, b] | padding_mask[None, :, b])  # [s, t]
            mT = keep2d.T.astype(BF16)  # [t, s]
            m["maskT"] = np.ascontiguousarray(mT.reshape(NT, P, S).transpose(1, 0, 2))
        in_maps.append(m)

    trace = bool(os.environ.get("BASS_KERNEL_TRACE"))
    res = run_bass_kernel_spmd(nc, in_maps, core_ids=list(range(N_CORES)),
                               trace=trace)
    LAST_RESULT = res

    out = np.empty((S, B, C), dtype=np.float32)
    for core in range(N_CORES):
        b, hg = core // 2, core % 2
        out[:, b, (C // 2) * hg:(C // 2) * (hg + 1)] = res.results[core]["out"]
    return out
